# revision 5
# baseline (speedup 1.0000x reference)
"""CLIP contrastive loss on 8 Trainium2 NeuronCores (Bass/Tile), fp8 DoubleRow.

Strategy (data-parallel over image rows, hint's local_loss path):
  - Core c holds image rows [c*1024, (c+1)*1024) and the FULL text matrix.
  - Text rows are rolled by c*1024 on the host so every core's diagonal
    block sits at local cols [0, 1024) (the compiled program is
    core-independent).
  - Features are scaled by 16 on the host and quantized to fp8 e4m3; the
    PE runs DoubleRow matmuls (2 fp8 k-planes per cell, K=256 per MM) at
    ~2x bf16 throughput.  PSUM accumulates exact f32; logits = psum *
    (scale/256).
  - Loop: for each 2048-col group (4 of them), for each 128-row m-tile
    (8): 2 kc x (2048/MM_W) DoubleRow MMs -> [128, 2048] PSUM (4 banks),
    then ONE ACT exp over the whole span (bf16 out, accum_out = partial
    row sums), and a DVE add into the group's column accumulator.
  - Diagonals all live in group 0 (cols mt*128..mt*128+128): DVE
    tensor_mul with (scale/256)*I + free-axis reduce.
  - Host: partition-reduce column accumulators, combine row/col exp-sums
    and diagonals in float64: lse = shift + log(sum); mean over both
    directions.

Fixed-shift logsumexp is numerically safe: logits are bounded by +-scale
and shift = scale/2 keeps every term that matters in normal f32 range.
"""

from contextlib import ExitStack

import numpy as np
import ml_dtypes

import concourse.bass as bass
from concourse import bacc
import concourse.tile as tile
from concourse import mybir
from concourse.bass import ts
from concourse.bass_utils import run_bass_kernel_spmd

N = 8192
D = 512
NC = 8
M_LOC = N // NC          # 1024 image rows per core
MT = M_LOC // 128        # 8 m-tiles of 128 rows
NG = 4                   # column groups
GW = N // NG             # 2048 cols per group
KC = 2                   # DoubleRow K-chunks (256 each)
PRE = 16.0               # host-side fp8 pre-scale per operand

F32 = mybir.dt.float32
BF16 = mybir.dt.bfloat16
FP8 = mybir.dt.float8e4

# moving-free per matmul: out width = MM_W, moving fp8 elements = 2*MM_W
MM_W = 512
# (g, mt) slots whose exp runs on DVE (Schraudolph int16/bf16 bit trick)
# instead of ACT, to balance the two engines. None at mt==0 (colacc copy)
# or mt==7 (tail path).
OFFLOAD = {(0, 4), (1, 2), (1, 5), (2, 2), (2, 5), (3, 2), (3, 5)}
LOG2E_BF16 = 128.0 / 0.6931471805599453   # 2^7 / ln 2
SCHRAUDOLPH_C = 5.5

_CACHE = {}
LAST_RESULTS = None


def _build(scale: float, shift: float, mm_w: int):
    n_mm = GW // mm_w            # matmuls per (kc, group-span)
    act_scale = scale / (PRE * PRE)
    ts_a = act_scale * LOG2E_BF16
    ts_b = 127.0 * 128.0 - SCHRAUDOLPH_C - shift * LOG2E_BF16
    nc = bacc.Bacc("TRN2", debug=False)

    at_d = nc.dram_tensor("at_in", [128, KC, 2, M_LOC], FP8, kind="ExternalInput").ap()
    bt_d = nc.dram_tensor("bt_in", [NG, KC, 128, 2, GW], FP8, kind="ExternalInput").ap()
    eye_d = nc.dram_tensor("eye_in", [128, 128], F32, kind="ExternalInput").ap()

    rowpart_d = nc.dram_tensor("rowpart_out", [128, MT, NG], F32, kind="ExternalOutput").ap()
    colsum_d = nc.dram_tensor("colsum_out", [NG, 128, GW], BF16, kind="ExternalOutput").ap()
    diag_d = nc.dram_tensor("diag_out", [128, MT], F32, kind="ExternalOutput").ap()

    with ExitStack() as ctx:
        tc = ctx.enter_context(tile.TileContext(nc))
        singles = ctx.enter_context(tc.tile_pool(name="singles", bufs=1))
        btp = ctx.enter_context(tc.tile_pool(name="btp", bufs=NG * KC))
        expp = ctx.enter_context(tc.tile_pool(name="expp", bufs=3))
        cap = ctx.enter_context(tc.tile_pool(name="cap", bufs=2))
        scr = ctx.enter_context(tc.tile_pool(name="scr", bufs=2))
        psum = ctx.enter_context(tc.tile_pool(name="psum", bufs=2, space="PSUM"))

        at_t = singles.tile([128, KC, 2, M_LOC], FP8)
        bt_tiles = [
            [btp.tile([128, 2, GW], FP8, name=f"bt{g}_{kc}", tag="bt") for kc in range(KC)]
            for g in range(NG)
        ]
        # Load order tuned for fastest first-matmul: at_kc0, then group 0's
        # kc0 text in two 1024-col chunks (the first matmuls only need the
        # first chunk), then the rest in consumption order.
        nc.sync.dma_start(at_t[:, 0], at_d[:, 0])
        for h in range(2):
            nc.sync.dma_start(
                bt_tiles[0][0][:, :, ts(h, GW // 2)], bt_d[0, 0][:, :, ts(h, GW // 2)]
            )
        nc.sync.dma_start(at_t[:, 1], at_d[:, 1])
        for h in range(2):
            nc.sync.dma_start(
                bt_tiles[0][1][:, :, ts(h, GW // 2)], bt_d[0, 1][:, :, ts(h, GW // 2)]
            )
        eye_t = singles.tile([128, 128], F32)
        nc.sync.dma_start(eye_t, eye_d)
        bias_t = singles.tile([128, 1], F32)
        nc.vector.memset(bias_t, -shift)

        rowpart = singles.tile([128, MT, NG], F32)
        diag_sb = singles.tile([128, MT], F32)

        for g in range(1, NG):
            for kc in range(KC):
                nc.sync.dma_start(bt_tiles[g][kc], bt_d[g, kc])

        for g in range(NG):
            colacc = cap.tile([128, GW], BF16, name=f"cacc{g}", tag="cacc")
            for mt in range(MT):
                s_ps = psum.tile([128, GW], F32, name=f"s{g}_{mt}", tag="spsum")
                for kc in range(KC):
                    lhsT = at_t[:, kc, :, ts(mt, 128)]          # [128, 2, 128]
                    for w in range(n_mm):
                        nc.tensor.matmul(
                            s_ps[:, ts(w, mm_w)],
                            lhsT,
                            bt_tiles[g][kc][:, :, ts(w, mm_w)],  # [128, 2, mm_w]
                            start=(kc == 0),
                            stop=(kc == KC - 1),
                            perf_mode=mybir.MatmulPerfMode.DoubleRow,
                        )
                if g == 0:
                    # diag block for mt sits at local cols [mt*128, mt*128+128)
                    dscr = scr.tile([128, 128], F32, name=f"dscr{mt}", tag="dscr")
                    nc.vector.tensor_mul(dscr, s_ps[:, ts(mt, 128)], eye_t)
                    nc.vector.tensor_reduce(
                        out=diag_sb[:, mt : mt + 1],
                        in_=dscr,
                        axis=mybir.AxisListType.X,
                        op=mybir.AluOpType.add,
                    )
                if (g, mt) in OFFLOAD:
                    # DVE bit-trick exp: y_bits = int16(x*A + B) viewed as
                    # bf16 is ~exp(logit - shift) to ~1.5%; errors average
                    # out in the sums.
                    e_i16 = expp.tile([128, GW], mybir.dt.int16, name=f"ei{g}_{mt}", tag="exp")
                    nc.vector.tensor_scalar(
                        out=e_i16,
                        in0=s_ps,
                        scalar1=ts_a,
                        scalar2=ts_b,
                        op0=mybir.AluOpType.mult,
                        op1=mybir.AluOpType.add,
                    )
                    e_bf = e_i16.bitcast(BF16)
                    nc.vector.tensor_add(colacc, colacc, e_bf)
                    nc.vector.tensor_reduce(
                        out=rowpart[:, mt, g : g + 1],
                        in_=e_bf,
                        axis=mybir.AxisListType.X,
                        op=mybir.AluOpType.add,
                    )
                else:
                    e_t = expp.tile([128, GW], BF16, name=f"e{g}_{mt}", tag="exp")
                    nc.scalar.activation(
                        e_t,
                        s_ps,
                        mybir.ActivationFunctionType.Exp,
                        bias=bias_t,
                        scale=act_scale,
                        accum_out=rowpart[:, mt, g : g + 1],
                    )
                    if mt == 0:
                        nc.vector.tensor_copy(colacc, e_t)
                    else:
                        nc.vector.tensor_add(colacc, colacc, e_t)
            nc.sync.dma_start(colsum_d[g], colacc)

        nc.sync.dma_start(rowpart_d, rowpart)
        nc.sync.dma_start(diag_d, diag_sb)

    nc.compile()
    return nc


def _prep_inputs(img, txt, scale):
    fp8 = ml_dtypes.float8_e4m3fn
    eye = ((scale / (PRE * PRE)) * np.eye(128)).astype(np.float32)
    in_maps = []
    for c in range(NC):
        A = (PRE * img[c * M_LOC : (c + 1) * M_LOC]).astype(fp8)   # [1024, 512]
        # k = kc*256 + ko*128 + p
        at = np.ascontiguousarray(
            A.T.reshape(KC, 2, 128, M_LOC).transpose(2, 0, 1, 3)
        )                                                          # [128, KC, 2, 1024]
        tr = np.roll(txt, -c * M_LOC, axis=0)                      # local col j -> global (j + c*1024) % N
        B = (PRE * tr).astype(fp8)                                 # [8192, 512]
        bt = np.ascontiguousarray(
            B.T.reshape(KC, 2, 128, NG, GW).transpose(3, 0, 2, 1, 4)
        )                                                          # [NG, KC, 128, 2, GW]
        in_maps.append({"at_in": at, "bt_in": bt, "eye_in": eye})
    return in_maps


def kernel(image_features, text_features, logit_scale):
    global LAST_RESULTS
    img = np.ascontiguousarray(np.asarray(image_features, dtype=np.float32))
    txt = np.ascontiguousarray(np.asarray(text_features, dtype=np.float32))
    scale = float(np.asarray(logit_scale))
    shift = 0.5 * scale

    key = (scale, MM_W)
    if key not in _CACHE:
        _CACHE[key] = _build(scale, shift, MM_W)
    nc = _CACHE[key]

    in_maps = _prep_inputs(img, txt, scale)
    res = run_bass_kernel_spmd(nc, in_maps, core_ids=list(range(NC)))
    LAST_RESULTS = res

    colsum_tot = np.zeros(N, dtype=np.float64)
    lse_rows = []
    diags = []
    for c, r in enumerate(res.results):
        rowsum = r["rowpart_out"].astype(np.float64).sum(axis=2)    # [128, MT]
        lse_rows.append(shift + np.log(rowsum.T.reshape(-1)))       # row = mt*128 + p
        diags.append(r["diag_out"].astype(np.float64).T.reshape(-1))
        colsum_tot += np.roll(
            r["colsum_out"].astype(np.float64).sum(axis=1).reshape(-1), c * M_LOC
        )
    lse_row = np.concatenate(lse_rows)
    diag = np.concatenate(diags)
    lse_col = shift + np.log(colsum_tot)

    loss = 0.5 * (np.mean(lse_row - diag) + np.mean(lse_col - diag))
    return np.float32(loss)


# revision 6
# speedup vs baseline: 1.1211x; 1.1211x over previous
"""CLIP contrastive loss on 8 Trainium2 NeuronCores (Bass/Tile), fp8 DoubleRow.

Strategy (data-parallel over image rows, hint's local_loss path):
  - Core c holds image rows [c*1024, (c+1)*1024) and the FULL text matrix.
  - Text rows are rolled by c*1024 on the host so every core's diagonal
    block sits at local cols [0, 1024) (the compiled program is
    core-independent).
  - Features are scaled by 16 on the host and quantized to fp8 e4m3; the
    PE runs DoubleRow matmuls (2 fp8 k-planes per cell, K=256 per MM) at
    ~2x bf16 throughput.  PSUM accumulates exact f32; logits = psum *
    (scale/256).
  - Loop: for each 2048-col group (4 of them), for each 128-row m-tile
    (8): 2 kc x (2048/MM_W) DoubleRow MMs -> [128, 2048] PSUM (4 banks),
    then ONE ACT exp over the whole span (bf16 out, accum_out = partial
    row sums), and a DVE add into the group's column accumulator.
  - Diagonals all live in group 0 (cols mt*128..mt*128+128): DVE
    tensor_mul with (scale/256)*I + free-axis reduce.
  - Host: partition-reduce column accumulators, combine row/col exp-sums
    and diagonals in float64: lse = shift + log(sum); mean over both
    directions.

Fixed-shift logsumexp is numerically safe: logits are bounded by +-scale
and shift = scale/2 keeps every term that matters in normal f32 range.
"""

from contextlib import ExitStack

import numpy as np
import ml_dtypes

import concourse.bass as bass
from concourse import bacc
import concourse.tile as tile
from concourse import mybir
from concourse.bass import ts
from concourse.bass_utils import run_bass_kernel_spmd

N = 8192
D = 512
NC = 8
M_LOC = N // NC          # 1024 image rows per core
MT = M_LOC // 128        # 8 m-tiles of 128 rows
NG = 4                   # column groups
GW = N // NG             # 2048 cols per group
KC = 2                   # DoubleRow K-chunks (256 each)
PRE = 16.0               # host-side fp8 pre-scale per operand

F32 = mybir.dt.float32
BF16 = mybir.dt.bfloat16
FP8 = mybir.dt.float8e4

# moving-free per matmul: out width = MM_W, moving fp8 elements = 2*MM_W
MM_W = 512
# (g, mt) slots whose exp runs on DVE (Schraudolph int16/bf16 bit trick)
# instead of ACT, to balance the two engines. None at mt==0 (colacc copy)
# or mt==7 (tail path).
OFFLOAD = {(1, 2), (1, 5), (2, 3), (3, 2), (3, 5)}
LOG2E_BF16 = 128.0 / 0.6931471805599453   # 2^7 / ln 2
SCHRAUDOLPH_C = 5.5

_CACHE = {}
LAST_RESULTS = None


def _build(scale: float, shift: float, mm_w: int):
    n_mm = GW // mm_w            # matmuls per (kc, group-span)
    act_scale = scale / (PRE * PRE)
    ts_a = act_scale * LOG2E_BF16
    ts_b = 127.0 * 128.0 - SCHRAUDOLPH_C - shift * LOG2E_BF16
    nc = bacc.Bacc("TRN2", debug=False)

    at_d = nc.dram_tensor("at_in", [128, KC, 2, M_LOC], FP8, kind="ExternalInput").ap()
    bt_d = nc.dram_tensor("bt_in", [NG, KC, 128, 2, GW], FP8, kind="ExternalInput").ap()
    eye_d = nc.dram_tensor("eye_in", [128, 128], F32, kind="ExternalInput").ap()

    rowpart_d = nc.dram_tensor("rowpart_out", [128, MT, NG], F32, kind="ExternalOutput").ap()
    colsum_d = nc.dram_tensor("colsum_out", [NG, 128, GW], BF16, kind="ExternalOutput").ap()
    diag_d = nc.dram_tensor("diag_out", [128, MT], F32, kind="ExternalOutput").ap()

    with ExitStack() as ctx:
        tc = ctx.enter_context(tile.TileContext(nc))
        singles = ctx.enter_context(tc.tile_pool(name="singles", bufs=1))
        btp = ctx.enter_context(tc.tile_pool(name="btp", bufs=NG * KC))
        expp = ctx.enter_context(tc.tile_pool(name="expp", bufs=3))
        cap = ctx.enter_context(tc.tile_pool(name="cap", bufs=2))
        scr = ctx.enter_context(tc.tile_pool(name="scr", bufs=2))
        psum = ctx.enter_context(tc.tile_pool(name="psum", bufs=2, space="PSUM"))

        at_t = singles.tile([128, KC, 2, M_LOC], FP8)
        bt_tiles = [
            [btp.tile([128, 2, GW], FP8, name=f"bt{g}_{kc}", tag="bt") for kc in range(KC)]
            for g in range(NG)
        ]
        # Load order tuned for fastest first-matmul: at_kc0, then group 0's
        # kc0 text in two 1024-col chunks (the first matmuls only need the
        # first chunk), then the rest in consumption order.
        nc.sync.dma_start(at_t[:, 0], at_d[:, 0])
        for h in range(2):
            nc.sync.dma_start(
                bt_tiles[0][0][:, :, ts(h, GW // 2)], bt_d[0, 0][:, :, ts(h, GW // 2)]
            )
        nc.sync.dma_start(at_t[:, 1], at_d[:, 1])
        for h in range(2):
            nc.sync.dma_start(
                bt_tiles[0][1][:, :, ts(h, GW // 2)], bt_d[0, 1][:, :, ts(h, GW // 2)]
            )
        eye_t = singles.tile([128, 128], F32)
        nc.sync.dma_start(eye_t, eye_d)
        bias_t = singles.tile([128, 1], F32)
        nc.vector.memset(bias_t, -shift)

        rowpart = singles.tile([128, MT, NG], F32)
        diag_sb = singles.tile([128, MT], F32)

        for g in range(1, NG):
            for kc in range(KC):
                nc.sync.dma_start(bt_tiles[g][kc], bt_d[g, kc])

        for g in range(NG):
            colacc = cap.tile([128, GW], BF16, name=f"cacc{g}", tag="cacc")
            for mt in range(MT):
                s_ps = psum.tile([128, GW], F32, name=f"s{g}_{mt}", tag="spsum")
                for kc in range(KC):
                    lhsT = at_t[:, kc, :, ts(mt, 128)]          # [128, 2, 128]
                    for w in range(n_mm):
                        nc.tensor.matmul(
                            s_ps[:, ts(w, mm_w)],
                            lhsT,
                            bt_tiles[g][kc][:, :, ts(w, mm_w)],  # [128, 2, mm_w]
                            start=(kc == 0),
                            stop=(kc == KC - 1),
                            perf_mode=mybir.MatmulPerfMode.DoubleRow,
                        )
                if g == 0:
                    # diag block for mt sits at local cols [mt*128, mt*128+128)
                    dscr = scr.tile([128, 128], F32, name=f"dscr{mt}", tag="dscr")
                    nc.vector.tensor_mul(dscr, s_ps[:, ts(mt, 128)], eye_t)
                    nc.vector.tensor_reduce(
                        out=diag_sb[:, mt : mt + 1],
                        in_=dscr,
                        axis=mybir.AxisListType.X,
                        op=mybir.AluOpType.add,
                    )
                if (g, mt) in OFFLOAD:
                    # DVE bit-trick exp: y_bits = int16(x*A + B) viewed as
                    # bf16 is ~exp(logit - shift) to ~1.5%; errors average
                    # out in the sums.
                    e_i16 = expp.tile([128, GW], mybir.dt.int16, name=f"ei{g}_{mt}", tag="exp")
                    nc.vector.tensor_scalar(
                        out=e_i16,
                        in0=s_ps,
                        scalar1=ts_a,
                        scalar2=ts_b,
                        op0=mybir.AluOpType.mult,
                        op1=mybir.AluOpType.add,
                    )
                    e_bf = e_i16.bitcast(BF16)
                    nc.vector.tensor_add(colacc, colacc, e_bf)
                    nc.vector.tensor_reduce(
                        out=rowpart[:, mt, g : g + 1],
                        in_=e_bf,
                        axis=mybir.AxisListType.X,
                        op=mybir.AluOpType.add,
                    )
                else:
                    e_t = expp.tile([128, GW], BF16, name=f"e{g}_{mt}", tag="exp")
                    nc.scalar.activation(
                        e_t,
                        s_ps,
                        mybir.ActivationFunctionType.Exp,
                        bias=bias_t,
                        scale=act_scale,
                        accum_out=rowpart[:, mt, g : g + 1],
                    )
                    if mt == 0:
                        nc.vector.tensor_copy(colacc, e_t)
                    else:
                        nc.vector.tensor_add(colacc, colacc, e_t)
            nc.sync.dma_start(colsum_d[g], colacc)

        nc.sync.dma_start(rowpart_d, rowpart)
        nc.sync.dma_start(diag_d, diag_sb)

    nc.compile()
    return nc


def _prep_inputs(img, txt, scale):
    fp8 = ml_dtypes.float8_e4m3fn
    eye = ((scale / (PRE * PRE)) * np.eye(128)).astype(np.float32)
    in_maps = []
    for c in range(NC):
        A = (PRE * img[c * M_LOC : (c + 1) * M_LOC]).astype(fp8)   # [1024, 512]
        # k = kc*256 + ko*128 + p
        at = np.ascontiguousarray(
            A.T.reshape(KC, 2, 128, M_LOC).transpose(2, 0, 1, 3)
        )                                                          # [128, KC, 2, 1024]
        tr = np.roll(txt, -c * M_LOC, axis=0)                      # local col j -> global (j + c*1024) % N
        B = (PRE * tr).astype(fp8)                                 # [8192, 512]
        bt = np.ascontiguousarray(
            B.T.reshape(KC, 2, 128, NG, GW).transpose(3, 0, 2, 1, 4)
        )                                                          # [NG, KC, 128, 2, GW]
        in_maps.append({"at_in": at, "bt_in": bt, "eye_in": eye})
    return in_maps


def kernel(image_features, text_features, logit_scale):
    global LAST_RESULTS
    img = np.ascontiguousarray(np.asarray(image_features, dtype=np.float32))
    txt = np.ascontiguousarray(np.asarray(text_features, dtype=np.float32))
    scale = float(np.asarray(logit_scale))
    shift = 0.5 * scale

    key = (scale, MM_W)
    if key not in _CACHE:
        _CACHE[key] = _build(scale, shift, MM_W)
    nc = _CACHE[key]

    in_maps = _prep_inputs(img, txt, scale)
    res = run_bass_kernel_spmd(nc, in_maps, core_ids=list(range(NC)))
    LAST_RESULTS = res

    colsum_tot = np.zeros(N, dtype=np.float64)
    lse_rows = []
    diags = []
    for c, r in enumerate(res.results):
        rowsum = r["rowpart_out"].astype(np.float64).sum(axis=2)    # [128, MT]
        lse_rows.append(shift + np.log(rowsum.T.reshape(-1)))       # row = mt*128 + p
        diags.append(r["diag_out"].astype(np.float64).T.reshape(-1))
        colsum_tot += np.roll(
            r["colsum_out"].astype(np.float64).sum(axis=1).reshape(-1), c * M_LOC
        )
    lse_row = np.concatenate(lse_rows)
    diag = np.concatenate(diags)
    lse_col = shift + np.log(colsum_tot)

    loss = 0.5 * (np.mean(lse_row - diag) + np.mean(lse_col - diag))
    return np.float32(loss)
